# revision 22
# baseline (speedup 1.0000x reference)
"""BitLinear (ternary-weight linear) on 8 Trainium2 NeuronCores.

Computation: out = x @ (clip(round(w/s), -1, 1) * s).T + bias, where s is
the per-output-row lower median of |w|.

Strategy
- Host side: compute the per-row scale s (exact reference semantics via
  np.partition) and the ternary weights wq in {-1, 0, 1}. The scale is
  applied in the on-device epilogue.
- Matmul dtype: fp8 e4m3 with MatmulPerfMode.DoubleRow (two 128-deep
  k-tiles contracted per instruction; the PE streams the doubled rhs at
  2 fp8 rows/cycle, i.e. 2x the MACs/cycle of bf16/f32r, 157 TF/s
  measured). Ternary weights are exact in e4m3; x is sent as a
  two-term decomposition x ~= x8 + r8/16 (x8 = e4m3(x),
  r8 = e4m3(16*(x - x8))), and the matmul runs as one augmented GEMM
    out = [x8 | r8[:, :KR*128]] @ [Wq ; Wq[:KR*128]/16]
  over KAUG = 32 + KR k-tiles. Wq/16 is ternary * 2^-4, still exact in
  e4m3. The residual correction covers KR of the 32 native k-tiles;
  the uncorrected remainder leaves a deterministic ~2.65e-2*sqrt(1 -
  KR/32) relative error (KR=16 -> 1.88e-2, measured bit-exact against
  the hardware).
- Sharding: data-parallel over tokens. Each core owns 1024 of the 8192
  tokens; its augmented x-shard (6.3 MB fp8) sits resident in SBUF
  (double-buffered so the load pipelines across invocations) while the
  augmented weight matrix (25.1 MB fp8) streams through once. No
  collectives. x-loads, weight stream, and out stores ride separate DMA
  queues (scalar/gpsimd/sync) to avoid FIFO coupling.
- Per core: psum tiles [128 tokens x 512 features] x 8 banks accumulate
  over the augmented contraction. Drain order matters: all psum->sbuf
  scale-mults issue first on DVE (freeing psum banks for the next
  feature block's start=True matmuls), then bias-adds on gpsimd and
  stores. Measured 312971 ns vs the 312.7 us PE floor at 2.515 GHz.
"""

import os
import sys

import numpy as np

for _p in ("/opt/trn_rl_repo", "/opt/pypackages"):
    if os.path.isdir(_p) and _p not in sys.path:
        sys.path.append(_p)

N_CORES = 8
B, S, IN_F, OUT_F = 4, 2048, 4096, 4096
TOK = B * S                # 8192 tokens total
TPC = TOK // N_CORES       # 1024 tokens per core
KB = IN_F // 128           # 32 native contraction blocks
KR = 16                    # residual k-tiles (partial correction)
KAUG = KB + KR             # 48 augmented k-tiles (main + partial residual)
KP = KAUG // 2             # 24 DoubleRow k-tile pairs
FBW = 512                  # psum tile free width (one PSUM bank of fp32)
FB = OUT_F // FBW          # 8 feature blocks
TB = TPC // 128            # 8 token blocks per core
X_CHUNKS = KP              # DMAs used to land the resident x-shard

_CACHE = {}


def _patched_tile_context(nc):
    """TileContext subclass for this container's walrus, which rejects
    instructions carrying more than one sync-wait command. Tile's wait
    assignment (and its tail drain) can attach several; after scheduling,
    move the extras onto same-engine no-ops inserted just before the
    instruction (same program point, identical semantics)."""
    import concourse.mybir as mybir
    import concourse.tile as tile

    def _split_multi_waits(nc):
        for f in nc.m.functions:
            for blk in f.blocks:
                out = []
                changed = False
                for inst in blk.instructions:
                    si = inst.sync_info
                    waits = list(si.on_wait) if si and si.on_wait else []
                    cap = 2 if isinstance(inst, mybir.InstEventSemaphore) else 1
                    if len(waits) > cap:
                        changed = True
                        for w in waits[:-cap]:
                            nop = mybir.InstNoOp(
                                name=f"I-waitsplit-{nc.next_id()}", ins=[], outs=[]
                            )
                            nop.engine = inst.engine
                            nop.sync_info = mybir.SyncInfo(on_wait=[w], on_update=[])
                            out.append(nop)
                        inst.sync_info = mybir.SyncInfo(
                            on_wait=waits[-cap:], on_update=list(si.on_update or [])
                        )
                    out.append(inst)
                if changed:
                    blk.instructions = out

    class PatchedTileContext(tile.TileContext):
        def schedule_and_allocate(self):
            result = super().schedule_and_allocate()
            _split_multi_waits(self.nc)
            return result

    return PatchedTileContext(nc)


def _build_nc():
    import concourse.bass as bass
    import concourse.mybir as mybir

    F32 = mybir.dt.float32
    F8 = mybir.dt.float8e4

    nc = bass.Bass()
    xt = nc.declare_dram_parameter("xt", [128, KAUG, TPC], F8, isOutput=False)
    wq8 = nc.declare_dram_parameter("wq8", [KP * 128, 2, OUT_F], F8, isOutput=False)
    s_pc = nc.declare_dram_parameter("s_pc", [128, OUT_F // 128], F32, isOutput=False)
    b_pc = nc.declare_dram_parameter("b_pc", [128, OUT_F // 128], F32, isOutput=False)
    out = nc.declare_dram_parameter("out", [OUT_F, TPC], F32, isOutput=True)

    with _patched_tile_context(nc) as tc:
        with tc.tile_pool(name="xp", bufs=2) as xp, \
             tc.tile_pool(name="cp", bufs=1) as cp, \
             tc.tile_pool(name="wp", bufs=12) as wp, \
             tc.tile_pool(name="op", bufs=10) as op, \
             tc.tile_pool(name="pp", bufs=1, space="PSUM") as pp:

            xt_sb = xp.tile([128, KAUG, TPC], F8)
            ca = KAUG // X_CHUNKS
            for c in range(X_CHUNKS):
                nc.scalar.dma_start(
                    xt_sb[:, c * ca:(c + 1) * ca, :], xt[:, c * ca:(c + 1) * ca, :]
                )
            s_sb = cp.tile([128, OUT_F // 128], F32, name="s_sb")
            nc.sync.dma_start(s_sb[:], s_pc[:])
            b_sb = cp.tile([128, OUT_F // 128], F32, name="b_sb")
            nc.sync.dma_start(b_sb[:], b_pc[:])

            # Transposed orientation: weights are the stationary operand
            # (lhsT, a 128-feature column block of wt), x is the moving
            # operand. Each LDWEIGHTS serves the two 512-token matmuls, so
            # the DoubleRow 256-column weight load (~200 ns) hides under
            # ~300 ns of matmul streaming instead of gating every matmul.
            for g in range(FB):          # feature group: 4 f128-blocks
                ptiles = [
                    pp.tile([128, TPC // 2], F32, name=f"ps{f}{t}", tag=f"ps{f}{t}")
                    for f in range(4) for t in range(2)
                ]
                for kp in range(KP):
                    wt = wp.tile([128, 2, FBW], F8, name="wt", tag="wt")
                    nc.gpsimd.dma_start(
                        wt[:], wq8[kp * 128:(kp + 1) * 128, :, g * FBW:(g + 1) * FBW]
                    )
                    for f in range(4):
                        for t in range(2):
                            nc.tensor.matmul(
                                ptiles[2 * f + t][:],
                                lhsT=wt[:, :, f * 128:(f + 1) * 128],
                                rhs=xt_sb[:, 2 * kp:2 * kp + 2,
                                          t * (TPC // 2):(t + 1) * (TPC // 2)],
                                start=(kp == 0),
                                stop=(kp == KP - 1),
                                perf_mode=mybir.MatmulPerfMode.DoubleRow,
                            )
                # Drain: fused scale+bias per-partition in one DVE op per tile
                # (scale/bias are per-feature = per-partition here).
                for f in range(4):
                    fi = g * 4 + f
                    for t in range(2):
                        ot = op.tile([128, TPC // 2], F32, name="ot", tag="ot")
                        nc.vector.tensor_scalar(
                            ot[:], ptiles[2 * f + t][:],
                            s_sb[:, fi:fi + 1], b_sb[:, fi:fi + 1],
                            mybir.AluOpType.mult, mybir.AluOpType.add,
                        )
                        nc.sync.dma_start(
                            out[fi * 128:(fi + 1) * 128,
                                t * (TPC // 2):(t + 1) * (TPC // 2)],
                            ot[:],
                        )
    return nc


def _get_nc():
    if "nc" not in _CACHE:
        _CACHE["nc"] = _build_nc()
    return _CACHE["nc"]


def _get_runner():
    """Jitted SPMD executor for the prebuilt Bass module, traced once and
    cached. Mirrors concourse.bass2jax.run_bass_via_pjrt's multi-core
    path, but reusable across calls: inputs are global arrays sharded on
    axis 0 over the 8 cores; output zero-buffers are generated on-device
    and donated."""
    if "runner" in _CACHE:
        return _CACHE["runner"]
    import jax
    import jax.numpy as jnp
    from jax.experimental.shard_map import shard_map
    from jax.sharding import Mesh, NamedSharding, PartitionSpec

    import concourse.mybir as mybir
    from concourse import bass2jax

    nc = _get_nc()
    assert nc.dbg_addr is None
    bass2jax.install_neuronx_cc_hook()

    partition_name = (
        nc.partition_id_tensor.name if nc.partition_id_tensor else None
    )
    in_names, out_names, out_avals = [], [], []
    for alloc in nc.m.functions[0].allocations:
        if not isinstance(alloc, mybir.MemoryLocationSet):
            continue
        name = alloc.memorylocations[0].name
        if alloc.kind == "ExternalInput":
            if name != partition_name:
                in_names.append(name)
        elif alloc.kind == "ExternalOutput":
            out_names.append(name)
            out_avals.append(
                jax.core.ShapedArray(
                    tuple(alloc.tensor_shape), mybir.dt.np(alloc.dtype)
                )
            )
    n_params, n_outs = len(in_names), len(out_names)
    all_in_names = tuple(
        in_names + out_names + ([partition_name] if partition_name else [])
    )

    def _body(*args):
        operands = list(args)
        if partition_name is not None:
            operands.append(bass2jax.partition_id_tensor())
        outs = bass2jax._bass_exec_p.bind(
            *operands,
            out_avals=tuple(out_avals),
            in_names=all_in_names,
            out_names=tuple(out_names),
            lowering_input_output_aliases=(),
            sim_require_finite=True,
            sim_require_nnan=True,
            nc=nc,
        )
        return tuple(outs)

    devices = jax.devices()[:N_CORES]
    mesh = Mesh(np.asarray(devices), ("core",))
    sharding = NamedSharding(mesh, PartitionSpec("core"))
    in_specs = (PartitionSpec("core"),) * (n_params + n_outs)
    out_specs = (PartitionSpec("core"),) * n_outs
    donate = tuple(range(n_params, n_params + n_outs))
    sharded = jax.jit(
        shard_map(
            _body, mesh=mesh, in_specs=in_specs, out_specs=out_specs,
            check_rep=False,
        ),
        donate_argnums=donate,
        keep_unused=True,
    )
    zeros_fn = jax.jit(
        lambda: tuple(
            jnp.zeros((N_CORES * a.shape[0], *a.shape[1:]), a.dtype)
            for a in out_avals
        ),
        out_shardings=(sharding,) * n_outs,
    )
    runner = dict(
        in_names=in_names, out_names=out_names, sharded=sharded,
        zeros_fn=zeros_fn, sharding=sharding,
    )
    _CACHE["runner"] = runner
    return runner


def _prep_inputs(x, weight, bias):
    """Host-side quantization, fp8 two-term decomposition, layout, and
    per-core sharding. Returns the global (axis-0 core-sharded) input
    arrays in runner order."""
    import ml_dtypes

    F8 = ml_dtypes.float8_e4m3
    x = np.asarray(x, dtype=np.float32)
    weight = np.asarray(weight, dtype=np.float32)
    bias = np.asarray(bias, dtype=np.float32)

    # Ternary quantization (matches the reference bit-for-bit): per-row
    # lower median of |w|, floored at 1e-12; wq = clip(round(w/s), -1, 1).
    mid = (IN_F - 1) // 2
    s = np.partition(np.abs(weight), mid, axis=1)[:, mid]
    s = np.maximum(s, np.float32(1e-12)).astype(np.float32)
    wq = np.clip(np.round(weight / s[:, None]), -1.0, 1.0).astype(np.float32)

    # Augmented weights: [Wq ; Wq[:KR*128]/16] along in_f, exact in e4m3.
    # Device layout [kp, p, term, f] where ka = 2*kp + term.
    waug = np.concatenate(
        [wq.T, wq.T[:KR * 128] * np.float32(1.0 / 16.0)], axis=0
    )
    w8 = waug.astype(F8).reshape(KAUG, 128, OUT_F)
    w8 = np.ascontiguousarray(
        w8.reshape(KP, 2, 128, OUT_F).transpose(0, 2, 1, 3)
    ).reshape(KP * 128, 2, OUT_F)

    # Per-partition scale/bias for the transposed (feature-partition) psum:
    # s_h[p, fi] = s[fi*128 + p].
    s_h = np.ascontiguousarray(s.reshape(OUT_F // 128, 128).T)
    b_h = np.ascontiguousarray(bias.reshape(OUT_F // 128, 128).T)

    # Two-term x decomposition: x ~= x8 + r8/16, both e4m3.
    xf = x.reshape(TOK, IN_F)
    x8 = xf.astype(F8)
    r8 = ((xf - x8.astype(np.float32)) * np.float32(16.0)).astype(F8)
    # Augmented per-core shard, laid out [partition=i%128, ka, tok] with
    # ka in [0, KB) the main tiles and [KB, KAUG) the residual tiles.
    xaug = np.concatenate([x8, r8[:, :KR * 128]], axis=1)  # [TOK, KAUG*128]
    x4 = xaug.reshape(N_CORES, TPC, KAUG, 128)
    xt_all = np.ascontiguousarray(x4.transpose(0, 3, 2, 1)).reshape(
        N_CORES * 128, KAUG, TPC
    )
    per_name = {
        "xt": xt_all,
        "wq8": np.broadcast_to(w8, (N_CORES, KP * 128, 2, OUT_F)).reshape(
            N_CORES * KP * 128, 2, OUT_F
        ),
        "s_pc": np.broadcast_to(s_h, (N_CORES, 128, OUT_F // 128)).reshape(
            N_CORES * 128, OUT_F // 128
        ),
        "b_pc": np.broadcast_to(b_h, (N_CORES, 128, OUT_F // 128)).reshape(
            N_CORES * 128, OUT_F // 128
        ),
    }
    runner = _get_runner()
    return [np.ascontiguousarray(per_name[n]) for n in runner["in_names"]]


def _execute(dev_or_np_inputs):
    runner = _get_runner()
    zeros = runner["zeros_fn"]()
    outs = runner["sharded"](*dev_or_np_inputs, *zeros)
    return outs


def kernel(x, weight, bias):
    global_inputs = _prep_inputs(x, weight, bias)
    outs = _execute(global_inputs)
    out_name_idx = _get_runner()["out_names"].index("out")
    # [N_CORES*OUT_F, TPC]: per-core feature-major blocks; un-transpose.
    out = np.asarray(outs[out_name_idx]).reshape(N_CORES, OUT_F, TPC)
    out = np.ascontiguousarray(out.transpose(0, 2, 1)).reshape(B, S, OUT_F)
    return out
